# revision 1
# baseline (speedup 1.0000x reference)
"""Trainium2 Bass kernel for nn_CorrClassLoss.

Reference computation (B=4, C=19, H=512, W=1024, N=5000, IGNORE=255):
  ref_class = argmax_c inputs_ref[b].reshape(C, H*W)      # flat W-major
  lin_ref   = 512*y_ref + x_ref    (NOTE: linearized with H, kept faithfully)
  lin_other = 512*y_other + x_other
  gathered  = ref_class[b, lin_ref]
  target[b, lin_other] = gathered  (scatter, last write wins; rest IGNORE)
  loss = mean over non-ignored pixels of -log_softmax(inputs_other)[b, target, px]

Since lin = 512*y + x with x,y in [0,512), only flat positions [0, 262144)
are ever touched, and at most N unique scatter destinations per batch
contribute to the loss:

  loss = -(1/cnt) * sum over unique dests d (last writer j, src s_j) of
         [ x_other[b, cls(s_j), d] - ln(sum_c exp(x_other[b, c, d])) ]
  cls(s) = argmax_c x_ref[b, c, s],  cnt = total unique dests.

Strategy (8 cores, data-parallel over (batch, half-of-correspondences)):
  Host does index-only math (dedup last-wins, split j by the pixel-half of
  s_j, pack padded gather-offset tables) and hands each core pixel-major
  transposed shards ref_t[px, c] / other_t[px, c] (a layout/sharding choice;
  all value compute happens on device).
  Device per core: [128,19]-run indirect gathers fetch the ref vector at s_j
  and the other vector at d_j; compact argmax one-hot (grouped max + is_ge);
  term1 = sum_j onehot_j . other_vec_j;  term2 = sum_j ln(sum_c exp
  (other_vec_j[c])) over valid j.  Output [1, 2] = (term1, term2).
  Host: loss = -(sum_cores term1 - term2) / cnt.
"""

import sys

if "/opt/trn_rl_repo" not in sys.path:
    sys.path.insert(0, "/opt/trn_rl_repo")

import numpy as np

B, C, H, W = 4, 19, 512, 1024
HW = H * W                 # 524288
NPIX = 262144              # touched flat range [0, 262144)
NPIX_H = NPIX // 2         # 131072 source pixels per core
N = 5000
NCORES = 8

P = 128                    # partitions

PAD_OFF = 1 << 28          # out-of-bounds offset => gather skipped, stays 0

CG_MAIN = 2688             # typical per-core capacity (21 columns)
CG_FALLBACK = 5120         # guaranteed upper bound (40 columns)

_programs = {}


def _build_program(cg):
    import concourse.bass as bass
    import concourse.bacc as bacc
    import concourse.mybir as mybir
    import concourse.tile as tile

    cgg = cg // P              # gather columns

    nc = bacc.Bacc("TRN2", target_bir_lowering=False, debug=False,
                   num_devices=NCORES)

    # pixel-major transposed shards: ref_t[px, c], other_t[px, c]
    ref_t = nc.dram_tensor("ref_t", [NPIX_H, C], mybir.dt.float32,
                           kind="ExternalInput")
    other_t = nc.dram_tensor("other_t", [NPIX, C], mybir.dt.float32,
                             kind="ExternalInput")
    # gather offsets: s_local*19 / d*19; element j at [j%P, j//P];
    # padded with PAD_OFF (gather skipped, row stays 0)
    s_off = nc.dram_tensor("s_off", [P, cgg], mybir.dt.int32,
                           kind="ExternalInput")
    d_off = nc.dram_tensor("d_off", [P, cgg], mybir.dt.int32,
                           kind="ExternalInput")
    out = nc.dram_tensor("out", [1, 2], mybir.dt.float32,
                         kind="ExternalOutput")

    ref_flat19 = ref_t.rearrange("p c -> (p c)")[:, None]
    other_flat19 = other_t.rearrange("p c -> (p c)")[:, None]

    with tile.TileContext(nc) as tc:
        with (
            tc.tile_pool(name="gb", bufs=1) as gb,
            tc.tile_pool(name="cons", bufs=1) as cons,
            tc.tile_pool(name="psum", bufs=1, space="PSUM") as psum,
        ):
            ones = cons.tile([P, 1], mybir.dt.float32)
            nc.gpsimd.memset(ones[:], 1.0)

            so = gb.tile([P, cgg], mybir.dt.int32)
            nc.sync.dma_start(out=so[:], in_=s_off[:, :])
            do = gb.tile([P, cgg], mybir.dt.int32)
            nc.sync.dma_start(out=do[:], in_=d_off[:, :])
            # pad mask depends only on so: compute early, off the tail
            pm = gb.tile([P, cgg], mybir.dt.float32)
            nc.vector.tensor_scalar(
                out=pm[:], in0=so[:], scalar1=NPIX_H * 19, scalar2=None,
                op0=mybir.AluOpType.is_lt,
            )
            # R needs no zero-init: pad rows' garbage is annihilated by
            # eq2*R2 (R2 pad rows ARE zeroed) and the pm mask on term2
            R = gb.tile([P, cgg * 19], mybir.dt.float32)
            R2 = gb.tile([P, cgg * 19], mybir.dt.float32)
            nc.vector.memset(R2[:], 0.0)
            # all ref gathers first: the argmax one-hot chain then overlaps
            # the other-vector gather stream
            for col in range(cgg):
                nc.gpsimd.indirect_dma_start(
                    out=R[:, col * 19:(col + 1) * 19],
                    out_offset=None,
                    in_=ref_flat19,
                    in_offset=bass.IndirectOffsetOnAxis(
                        ap=so[:, col:col + 1], axis=0),
                    bounds_check=NPIX_H * 19 - 1,
                    oob_is_err=False,
                )

            Rv = R[:].rearrange("p (g c) -> p g c", c=19)

            m2 = gb.tile([P, cgg], mybir.dt.float32)
            nc.vector.tensor_reduce(out=m2[:], in_=Rv,
                                    axis=mybir.AxisListType.X,
                                    op=mybir.AluOpType.max)
            eq2 = gb.tile([P, cgg * 19], mybir.dt.float32)
            eq2v = eq2[:].rearrange("p (g c) -> p g c", c=19)
            nc.vector.tensor_tensor(
                out=eq2v, in0=Rv,
                in1=m2[:, :, None].to_broadcast([P, cgg, 19]),
                op=mybir.AluOpType.is_ge,
            )

            # other-vector gathers in column halves; the dependent compute
            # for each half issues as soon as that half has landed
            t1g = gb.tile([P, cgg], mybir.dt.float32)
            e2 = gb.tile([P, cgg * 19], mybir.dt.float32)
            S2 = gb.tile([P, cgg], mybir.dt.float32)
            h0 = cgg // 2
            for lo, hi in ((0, h0), (h0, cgg)):
                for col in range(lo, hi):
                    nc.gpsimd.indirect_dma_start(
                        out=R2[:, col * 19:(col + 1) * 19],
                        out_offset=None,
                        in_=other_flat19,
                        in_offset=bass.IndirectOffsetOnAxis(
                            ap=do[:, col:col + 1], axis=0),
                        bounds_check=NPIX * 19 - 1,
                        oob_is_err=False,
                    )
                w = hi - lo
                if w == 0:
                    continue
                sl = slice(lo * 19, hi * 19)
                slg = slice(lo, hi)
                nc.vector.tensor_tensor(out=eq2[:, sl], in0=eq2[:, sl],
                                        in1=R2[:, sl],
                                        op=mybir.AluOpType.mult)
                nc.vector.tensor_reduce(
                    out=t1g[:, slg],
                    in_=eq2[:, sl].rearrange("p (g c) -> p g c", c=19),
                    axis=mybir.AxisListType.X,
                    op=mybir.AluOpType.add,
                )
                nc.scalar.activation(e2[:, sl], R2[:, sl],
                                     mybir.ActivationFunctionType.Exp)
                nc.vector.tensor_reduce(
                    out=S2[:, slg],
                    in_=e2[:, sl].rearrange("p (g c) -> p g c", c=19),
                    axis=mybir.AxisListType.X, op=mybir.AluOpType.add)
            L2 = gb.tile([P, cgg], mybir.dt.float32)
            nc.scalar.activation(L2[:], S2[:],
                                 mybir.ActivationFunctionType.Ln)
            nc.vector.tensor_tensor(out=L2[:], in0=L2[:], in1=pm[:],
                                    op=mybir.AluOpType.mult)

            # ---- combine ----------------------------------------------
            t1p = gb.tile([P, 1], mybir.dt.float32)
            nc.vector.tensor_reduce(out=t1p[:], in_=t1g[:],
                                    axis=mybir.AxisListType.X,
                                    op=mybir.AluOpType.add)
            t2p = gb.tile([P, 1], mybir.dt.float32)
            nc.vector.tensor_reduce(out=t2p[:], in_=L2[:],
                                    axis=mybir.AxisListType.X,
                                    op=mybir.AluOpType.add)
            pout = psum.tile([1, 2], mybir.dt.float32, space="PSUM")
            nc.tensor.matmul(out=pout[:, 0:1], lhsT=t1p[:], rhs=ones[:],
                             start=True, stop=True)
            nc.tensor.matmul(out=pout[:, 1:2], lhsT=t2p[:], rhs=ones[:],
                             start=True, stop=True)
            so_out = cons.tile([1, 2], mybir.dt.float32)
            nc.vector.tensor_copy(out=so_out[:], in_=pout[:])
            nc.sync.dma_start(out=out[:, :], in_=so_out[:])

    nc.finalize()
    return nc


def _get_program(cg):
    if cg not in _programs:
        _programs[cg] = _build_program(cg)
    return _programs[cg]


def _host_prep(inds_ref, inds_other):
    """Index-only host math: dedup scatter (last wins), partition per core."""
    ir = np.asarray(inds_ref).astype(np.int64)      # [B, 2, N]
    io = np.asarray(inds_other).astype(np.int64)
    valid = ((ir[:, 0] >= 0) & (ir[:, 0] < W) & (ir[:, 1] >= 0) & (ir[:, 1] < H)
             & (io[:, 0] >= 0) & (io[:, 0] < W) & (io[:, 1] >= 0)
             & (io[:, 1] < H))                       # [B, N]
    lin_ref = H * ir[:, 1] + ir[:, 0]                # [B, N]
    lin_other = H * io[:, 1] + io[:, 0]

    per_core = []
    count = 0
    need_fallback = False
    for b in range(B):
        v = valid[b]
        lo = lin_other[b][v]
        lr = np.clip(lin_ref[b][v], 0, HW - 1)
        # last-write-wins dedup on destinations
        u, first_rev = np.unique(lo[::-1], return_index=True)
        last_idx = len(lo) - 1 - first_rev
        d_arr = u.astype(np.int64)
        s_arr = lr[last_idx].astype(np.int64)
        count += len(u)
        for h in range(2):
            sel = (s_arr // NPIX_H) == h
            s_local = s_arr[sel] - h * NPIX_H
            d_sel = d_arr[sel]
            per_core.append({
                "b": b, "h": h,
                "s": s_local, "d": d_sel,
            })
    return per_core, count


def _pack_core(pc, cg):
    cgg = cg // P
    s_off = np.full((P, cgg), PAD_OFF, dtype=np.int32)
    d_off = np.full((P, cgg), PAD_OFF, dtype=np.int32)
    s, d = pc["s"], pc["d"]
    n = len(s)
    assert n <= cg
    jj = np.arange(n)
    s_off[jj % P, jj // P] = s * 19
    d_off[jj % P, jj // P] = d * 19
    return s_off, d_off


def _make_in_maps(inputs_ref, inputs_other, per_core, cg):
    ref_flat = inputs_ref.reshape(B, C, HW)
    other_flat = inputs_other.reshape(B, C, HW)
    # transposed shards; other_t shared by both cores of a batch pair
    other_cache = {}
    in_maps = []
    for pc in per_core:
        b, h = pc["b"], pc["h"]
        ref_td = np.ascontiguousarray(
            ref_flat[b, :, h * NPIX_H:(h + 1) * NPIX_H].T)
        if b not in other_cache:
            other_cache[b] = np.ascontiguousarray(other_flat[b, :, :NPIX].T)
        s_off, d_off = _pack_core(pc, cg)
        in_maps.append({
            "ref_t": ref_td,
            "other_t": other_cache[b],
            "s_off": s_off,
            "d_off": d_off,
        })
    return in_maps


def kernel(inputs_ref, inputs_other, inds_ref, inds_other, weights):
    from concourse.bass_utils import run_bass_kernel_spmd

    inputs_ref = np.asarray(inputs_ref, dtype=np.float32)
    inputs_other = np.asarray(inputs_other, dtype=np.float32)

    per_core, count = _host_prep(inds_ref, inds_other)
    # exact-fit capacity: compile (and cache) the program for the actual
    # worst-core correspondence count, rounded up to whole 128-columns
    max_n = max(len(pc["s"]) for pc in per_core)
    cg = max(128, -(-max_n // P) * P)
    nc = _get_program(cg)

    in_maps = _make_in_maps(inputs_ref, inputs_other, per_core, cg)
    res = run_bass_kernel_spmd(nc, in_maps, core_ids=list(range(NCORES)))
    total = 0.0
    for r in res.results:
        o = np.asarray(r["out"], dtype=np.float64)
        total += o[0, 0] - o[0, 1]
    loss = -total / max(count, 1)
    return np.float32(loss)



# revision 2
# speedup vs baseline: 5.0937x; 5.0937x over previous
"""Trainium2 Bass kernel for nn_CorrClassLoss.

Reference computation (B=4, C=19, H=512, W=1024, N=5000, IGNORE=255):
  ref_class = argmax_c inputs_ref[b].reshape(C, H*W)      # flat W-major
  lin_ref   = 512*y_ref + x_ref    (NOTE: linearized with H, kept faithfully)
  lin_other = 512*y_other + x_other
  gathered  = ref_class[b, lin_ref]
  target[b, lin_other] = gathered  (scatter, last write wins; rest IGNORE)
  loss = mean over non-ignored pixels of -log_softmax(inputs_other)[b, target, px]

Since lin = 512*y + x with x,y in [0,512), only flat positions [0, 262144)
are ever touched, and at most N unique scatter destinations per batch
contribute to the loss:

  loss = -(1/cnt) * sum over unique dests d (last writer j, src s_j) of
         [ x_other[b, cls(s_j), d] - ln(sum_c exp(x_other[b, c, d])) ]
  cls(s) = argmax_c x_ref[b, c, s],  cnt = total unique dests.

Strategy (8 cores, data-parallel over (batch, half-of-correspondences)):
  Host does index-only math (dedup last-wins, split j by the pixel-half of
  s_j, pack padded gather-offset tables) and hands each core one pixel-major
  transposed data block data_t = [ref_half_t; other_t] (a layout/sharding
  choice; all value compute happens on device).
  Device per core: ONE batched indirect gather fetches all ref vectors at
  s_j and other vectors at d_j (row offsets into data_t, 19 contiguous
  floats per row, ~5120 rows); compact argmax one-hot (grouped max +
  is_ge); term1 = sum_j onehot_j . other_vec_j; term2 = sum_j
  ln(sum_c exp(other_vec_j[c])) over valid j.  Output [2, 1] =
  (term1, term2).  Host: loss = -(sum_cores term1 - term2) / cnt.
"""

import sys

if "/opt/trn_rl_repo" not in sys.path:
    sys.path.insert(0, "/opt/trn_rl_repo")

import numpy as np

B, C, H, W = 4, 19, 512, 1024
HW = H * W                 # 524288
NPIX = 262144              # touched flat range [0, 262144)
NPIX_H = NPIX // 2         # 131072 source pixels per core
NROWS = NPIX_H + NPIX      # combined data block rows per core
N = 5000
NCORES = 8

P = 128                    # partitions

# out-of-bounds row offset => gather row skipped/zeroed; 19*PAD_OFF < 2^31
PAD_OFF = 1 << 26

_programs = {}


def _build_program(cg):
    import concourse.bass as bass
    import concourse.bacc as bacc
    import concourse.mybir as mybir
    import concourse.tile as tile

    cgg = cg // P              # gather groups per partition row (per stream)
    g2 = 2 * cgg               # ref groups + other groups

    nc = bacc.Bacc("TRN2", target_bir_lowering=False, debug=False,
                   num_devices=NCORES)

    # combined pixel-major block: rows [0, NPIX_H) = ref half shard,
    # rows [NPIX_H, NROWS) = full other image; 19 contiguous floats per row
    data_t = nc.dram_tensor("data_t", [NROWS, C], mybir.dt.float32,
                            kind="ExternalInput")
    # row offsets: element j at [j%P, j//P] (s) and [j%P, cgg + j//P]
    # (NPIX_H + d); padded with PAD_OFF
    offs = nc.dram_tensor("offs", [P, g2], mybir.dt.int32,
                          kind="ExternalInput")
    out = nc.dram_tensor("out", [2, 1], mybir.dt.float32,
                         kind="ExternalOutput")

    with tile.TileContext(nc) as tc:
        with (
            tc.tile_pool(name="gb", bufs=1) as gb,
            tc.tile_pool(name="cons", bufs=1) as cons,
            tc.tile_pool(name="psum", bufs=1, space="PSUM") as psum,
        ):
            ones = cons.tile([P, 1], mybir.dt.float32)
            nc.gpsimd.memset(ones[:], 1.0)

            so = gb.tile([P, g2], mybir.dt.int32)
            nc.sync.dma_start(out=so[:], in_=offs[:, :])

            # G holds all gathered vectors: groups [0, cgg) are ref vectors,
            # [cgg, 2cgg) are other vectors, 19 floats each
            G = gb.tile([P, g2 * C], mybir.dt.float32)
            # zero the other-vector half: pad rows must contribute exactly 0
            # to term1 (eq * 0) and a pm-masked ln(19) to term2
            nc.vector.memset(G[:, cgg * C:], 0.0)

            # one batched gather: 19 contiguous floats per offset row
            nc.gpsimd.indirect_dma_start(
                out=G[:, :],
                out_offset=None,
                in_=data_t[:, :],
                in_offset=bass.IndirectOffsetOnAxis(ap=so[:, :], axis=0),
                bounds_check=NROWS - 1,
                oob_is_err=False,
            )

            # pad mask off the d-offset half (pads are PAD_OFF >= NROWS)
            pm = gb.tile([P, cgg], mybir.dt.float32)
            nc.vector.tensor_scalar(
                out=pm[:], in0=so[:, cgg:], scalar1=NROWS, scalar2=None,
                op0=mybir.AluOpType.is_lt,
            )

            Rv = G[:, :cgg * C].rearrange("p (g c) -> p g c", c=C)
            R2 = G[:, cgg * C:]
            R2v = R2.rearrange("p (g c) -> p g c", c=C)

            # argmax one-hot of each ref vector: grouped max + is_ge
            m2 = gb.tile([P, cgg], mybir.dt.float32)
            nc.vector.tensor_reduce(out=m2[:], in_=Rv,
                                    axis=mybir.AxisListType.X,
                                    op=mybir.AluOpType.max)
            eq = gb.tile([P, cgg * C], mybir.dt.float32)
            eqv = eq[:].rearrange("p (g c) -> p g c", c=C)
            nc.vector.tensor_tensor(
                out=eqv, in0=Rv,
                in1=m2[:, :, None].to_broadcast([P, cgg, C]),
                op=mybir.AluOpType.is_ge,
            )
            # term1 per group: onehot . other_vec
            nc.vector.tensor_tensor(out=eq[:], in0=eq[:], in1=R2,
                                    op=mybir.AluOpType.mult)
            t1g = gb.tile([P, cgg], mybir.dt.float32)
            nc.vector.tensor_reduce(
                out=t1g[:], in_=eq[:].rearrange("p (g c) -> p g c", c=C),
                axis=mybir.AxisListType.X, op=mybir.AluOpType.add)

            # term2 per group: ln sum_c exp(other_vec)
            e2 = gb.tile([P, cgg * C], mybir.dt.float32)
            nc.scalar.activation(e2[:], R2,
                                 mybir.ActivationFunctionType.Exp)
            S2 = gb.tile([P, cgg], mybir.dt.float32)
            nc.vector.tensor_reduce(
                out=S2[:], in_=e2[:].rearrange("p (g c) -> p g c", c=C),
                axis=mybir.AxisListType.X, op=mybir.AluOpType.add)
            L2 = gb.tile([P, cgg], mybir.dt.float32)
            nc.scalar.activation(L2[:], S2[:],
                                 mybir.ActivationFunctionType.Ln)
            nc.vector.tensor_tensor(out=L2[:], in0=L2[:], in1=pm[:],
                                    op=mybir.AluOpType.mult)

            # ---- combine ----------------------------------------------
            T = gb.tile([P, 2], mybir.dt.float32)
            nc.vector.tensor_reduce(out=T[:, 0:1], in_=t1g[:],
                                    axis=mybir.AxisListType.X,
                                    op=mybir.AluOpType.add)
            nc.vector.tensor_reduce(out=T[:, 1:2], in_=L2[:],
                                    axis=mybir.AxisListType.X,
                                    op=mybir.AluOpType.add)
            pout = psum.tile([2, 1], mybir.dt.float32, space="PSUM")
            nc.tensor.matmul(out=pout[:], lhsT=T[:], rhs=ones[:],
                             start=True, stop=True)
            so_out = cons.tile([2, 1], mybir.dt.float32)
            nc.vector.tensor_copy(out=so_out[:], in_=pout[:])
            nc.sync.dma_start(out=out[:, :], in_=so_out[:])

    nc.finalize()
    return nc


def _get_program(cg):
    if cg not in _programs:
        _programs[cg] = _build_program(cg)
    return _programs[cg]


def _host_prep(inds_ref, inds_other):
    """Index-only host math: dedup scatter (last wins), partition per core."""
    ir = np.asarray(inds_ref).astype(np.int64)      # [B, 2, N]
    io = np.asarray(inds_other).astype(np.int64)
    valid = ((ir[:, 0] >= 0) & (ir[:, 0] < W) & (ir[:, 1] >= 0) & (ir[:, 1] < H)
             & (io[:, 0] >= 0) & (io[:, 0] < W) & (io[:, 1] >= 0)
             & (io[:, 1] < H))                       # [B, N]
    lin_ref = H * ir[:, 1] + ir[:, 0]                # [B, N]
    lin_other = H * io[:, 1] + io[:, 0]

    per_core = []
    count = 0
    for b in range(B):
        v = valid[b]
        lo = lin_other[b][v]
        lr = np.clip(lin_ref[b][v], 0, HW - 1)
        # last-write-wins dedup on destinations
        u, first_rev = np.unique(lo[::-1], return_index=True)
        last_idx = len(lo) - 1 - first_rev
        d_arr = u.astype(np.int64)
        s_arr = lr[last_idx].astype(np.int64)
        count += len(u)
        for h in range(2):
            sel = (s_arr // NPIX_H) == h
            s_local = s_arr[sel] - h * NPIX_H
            d_sel = d_arr[sel]
            per_core.append({
                "b": b, "h": h,
                "s": s_local, "d": d_sel,
            })
    return per_core, count


def _pack_core(pc, cg):
    cgg = cg // P
    offs = np.full((P, 2 * cgg), PAD_OFF, dtype=np.int32)
    s, d = pc["s"], pc["d"]
    n = len(s)
    assert n <= cg
    jj = np.arange(n)
    offs[jj % P, jj // P] = s
    offs[jj % P, cgg + jj // P] = NPIX_H + d
    return offs


def _make_in_maps(inputs_ref, inputs_other, per_core, cg):
    ref_flat = inputs_ref.reshape(B, C, HW)
    other_flat = inputs_other.reshape(B, C, HW)
    in_maps = []
    for pc in per_core:
        b, h = pc["b"], pc["h"]
        data = np.empty((NROWS, C), dtype=np.float32)
        data[:NPIX_H] = ref_flat[b, :, h * NPIX_H:(h + 1) * NPIX_H].T
        data[NPIX_H:] = other_flat[b, :, :NPIX].T
        in_maps.append({
            "data_t": data,
            "offs": _pack_core(pc, cg),
        })
    return in_maps


def kernel(inputs_ref, inputs_other, inds_ref, inds_other, weights):
    from concourse.bass_utils import run_bass_kernel_spmd

    inputs_ref = np.asarray(inputs_ref, dtype=np.float32)
    inputs_other = np.asarray(inputs_other, dtype=np.float32)

    per_core, count = _host_prep(inds_ref, inds_other)
    # exact-fit capacity: compile (and cache) the program for the actual
    # worst-core correspondence count, rounded up to whole 128-columns
    max_n = max(len(pc["s"]) for pc in per_core)
    cg = max(128, -(-max_n // P) * P)
    nc = _get_program(cg)

    in_maps = _make_in_maps(inputs_ref, inputs_other, per_core, cg)
    res = run_bass_kernel_spmd(nc, in_maps, core_ids=list(range(NCORES)))
    total = 0.0
    for r in res.results:
        o = np.asarray(r["out"], dtype=np.float64)
        total += o[0, 0] - o[1, 0]
    loss = -total / max(count, 1)
    return np.float32(loss)
